# revision 18
# baseline (speedup 1.0000x reference)
"""Multi-head attention (RoPE + length masking) on 8 Trainium2 NeuronCores.

Sharding: core c handles batch b = c // 4 and heads [4*(c%4), 4*(c%4)+4).
Each core computes q/k/v projections for its 256 inner dims, RoPE, per-head
attention with length masking, and a row-parallel slice of the output
projection; the host sums the 4 partial outputs per batch (all-reduce) and
applies the final key-mask zeroing.

Precision: float32r (reduced-mantissa fp32, full PE rate) for projections
and scores; bf16 for exp(scores), v, and the output projection. Measured
end-to-end relative error ~2e-3.
"""

import numpy as np
from contextlib import ExitStack

import bass_rust as _br
import concourse.bass as bass
import concourse.tile as tile
import concourse.mybir as mybir
from concourse.bass_utils import run_bass_kernel_spmd

P = 128
B, N, D_MODEL, H, DH = 2, 2048, 1024, 16, 64
N_CORES = 8
HPC = 4            # heads per core
CI = HPC * DH      # per-core inner dim (256)
KO = D_MODEL // P  # 8 contraction tiles

_counter = [0]


def _split_excess_waits(nc, max_normal=1, max_evsem=1):
    """Walrus in this toolchain rejects >1 semaphore wait per instruction.
    Hoist excess waits onto EventSemaphore carriers inserted right before
    the offending instruction in the same engine's program order."""
    for fn in nc.m.functions:
        for bb in fn.blocks:
            insts = bb.instructions
            out = []
            changed = False
            for inst in insts:
                si = inst.sync_info
                waits = list(si.on_wait or []) if si is not None else []
                cap = (
                    max_evsem
                    if isinstance(inst, mybir.InstEventSemaphore)
                    else max_normal
                )
                if len(waits) > cap:
                    extra, keep = waits[:-cap], waits[-cap:]
                    si.on_wait = keep
                    for i in range(0, len(extra), max_evsem):
                        _counter[0] += 1
                        ev = mybir.InstEventSemaphore(
                            name=f"I-waitsplit-{_counter[0]}",
                            engine=inst.engine,
                            sync_info=_br.SyncInfo(
                                on_wait=extra[i : i + max_evsem], on_update=[]
                            ),
                            ins=[],
                            outs=[],
                        )
                        out.append(ev)
                    changed = True
                out.append(inst)
            if changed:
                bb.instructions = out


class _TileContextFixed(tile.TileContext):
    def _drain_and_barrier(self, tick_clock, wait_clock):
        from concourse.tile import ScopedClock

        nc = self.nc
        drain_inst = nc.sync.drain()
        wait_clock.add_sem_waits(
            drain_inst.ins, ScopedClock({None: tick_clock.global_clock})
        )
        nc.all_engine_barrier()
        assert self.sems is not None
        popped = nc._tile_sem_poison_stack.pop()
        assert popped is self._sem_poison
        nc.clear_and_free_semaphores(list(self.sems.allocated().values()))
        nc.all_engine_barrier()
        _split_excess_waits(nc)


def _make_chunks(LK):
    """Split LK (multiple of 128) into free-dim chunks, preferring 512 and
    keeping every chunk >= 256 when possible (f32r full-rate needs >= 256)."""
    chunks = [512] * (LK // 512)
    rem = LK % 512
    if rem:
        if rem >= 256 or not chunks:
            chunks.append(rem)
        else:
            # e.g. rem=128 -> replace one 512 with 384 + 256
            chunks[-1] = 512 - (256 - rem)
            chunks.append(256)
    return chunks


def _build(LK):
    """Build the single-core Bass program (same program on all 8 cores)."""
    dt = mybir.dt
    KT = LK // P
    chunks = _make_chunks(LK)
    starts = np.cumsum([0] + chunks[:-1]).tolist()
    cidx = list(zip(chunks, starts))

    nc = bass.Bass(trn_type="TRN2")

    xT = nc.dram_tensor("xT", [D_MODEL, LK], dt.float32, kind="ExternalInput")
    wq = nc.dram_tensor("wq", [D_MODEL, CI], dt.float32, kind="ExternalInput")
    wk = nc.dram_tensor("wk", [D_MODEL, CI], dt.float32, kind="ExternalInput")
    wv = nc.dram_tensor("wv", [D_MODEL, CI], dt.float32, kind="ExternalInput")
    wo = nc.dram_tensor("wo", [CI, D_MODEL], dt.bfloat16, kind="ExternalInput")
    cosT = nc.dram_tensor("cosT", [CI, LK], dt.float32, kind="ExternalInput")
    sinT = nc.dram_tensor("sinT", [CI, LK], dt.float32, kind="ExternalInput")
    vmask = nc.dram_tensor("vmask", [P, KT], dt.float32, kind="ExternalInput")
    ones2 = nc.dram_tensor("ones2", [2, P], dt.float32, kind="ExternalInput")
    out = nc.dram_tensor("out", [LK, D_MODEL], dt.float32, kind="ExternalOutput")

    F = mybir.ActivationFunctionType
    ALU = mybir.AluOpType

    with _TileContextFixed(nc) as tc, ExitStack() as ctx:
        # ---- long-lived pools ----
        p_qk = ctx.enter_context(tc.tile_pool(name="qk", bufs=1))
        p_v = ctx.enter_context(tc.tile_pool(name="v", bufs=1))
        p_const = ctx.enter_context(tc.tile_pool(name="const", bufs=1))

        qT = p_qk.tile([P, 2, LK], dt.float32r)
        kT = p_qk.tile([P, 2, LK], dt.float32r)
        v_sb = p_v.tile([P, KT, HPC, DH + 1], dt.bfloat16)
        wo_sb = p_const.tile([P, 2, D_MODEL], dt.bfloat16)
        vm_sb = p_const.tile([P, KT], dt.float32)
        ones2_r = p_const.tile([2, P], dt.float32r)

        nc.sync.dma_start(vm_sb[:], vmask[:])
        nc.sync.dma_start(wo_sb[:], wo.rearrange("(po pi) f -> pi po f", pi=P))
        ones2_f = p_const.tile([2, P], dt.float32)
        nc.sync.dma_start(ones2_f[:], ones2[:])
        nc.vector.tensor_copy(ones2_r[:], ones2_f[:])
        # ones column of v_aug carries the k-mask (0 rows beyond length)
        nc.vector.tensor_copy(
            v_sb[:, :, :, DH],
            vm_sb[:, :, None].to_broadcast((P, KT, HPC)),
        )

        # ---- phase A: load x/W chunk-wise; project k, v; q is emitted
        #      per-chunk inside the attention loop for overlap ----
        if True:
            actx = ctx
            pa_w = actx.enter_context(tc.tile_pool(name="wstage", bufs=1))
            pa_x = actx.enter_context(tc.tile_pool(name="xtr", bufs=1))
            pa_stage = actx.enter_context(tc.tile_pool(name="stage", bufs=2))
            pa_tmp = actx.enter_context(tc.tile_pool(name="ropetmp", bufs=2))
            pa_cs = actx.enter_context(tc.tile_pool(name="cschunk", bufs=2))
            ps_proj = actx.enter_context(
                tc.tile_pool(name="psMix", bufs=2, space="PSUM")
            )

            cosT3 = cosT.rearrange("(po pi) s -> pi po s", pi=P)
            sinT3 = sinT.rearrange("(po pi) s -> pi po s", pi=P)
            w_r = {}
            wstg = {}
            for name, wdram in (("k", wk), ("v", wv), ("q", wq)):
                wst = pa_stage.tile([P, KO, CI], dt.float32, tag="wst", name="wst")
                nc.sync.dma_start(
                    wst[:], wdram.rearrange("(ko ki) c -> ki ko c", ki=P)
                )
                wstg[name] = wst
            for name in ("k", "v", "q"):
                wr = pa_w.tile([P, KO, CI], dt.float32r, tag=f"w{name}", name="wr")
                if name == "k":
                    nc.vector.tensor_copy(wr[:], wstg[name][:])
                else:
                    nc.gpsimd.tensor_copy(wr[:], wstg[name][:])
                w_r[name] = wr

            xTr = pa_x.tile([P, KO, LK], dt.float32r)
            xT3 = xT.rearrange("(ko ki) s -> ki ko s", ki=P)
            for cw, c0 in cidx:
                for ko in range(KO):
                    xst = pa_stage.tile(
                        [P, 512], dt.float32, tag="xst", name="xst"
                    )[:, :cw]
                    nc.sync.dma_start(xst[:, :], xT3[:, ko, c0 : c0 + cw])
                    eng = nc.vector if ko % 2 == 0 else nc.gpsimd
                    eng.tensor_copy(xTr[:, ko, c0 : c0 + cw], xst[:, :])

            def project_rope(name, dst, p, cw, c0):
                pp = ps_proj.tile([P, 512], dt.float32, tag="psm", name="pp")[:, :cw]
                for ko in range(KO):
                    nc.tensor.matmul(
                        pp[:, :],
                        w_r[name][:, ko, p * P : (p + 1) * P],
                        xTr[:, ko, c0 : c0 + cw],
                        start=(ko == 0),
                        stop=(ko == KO - 1),
                    )
                cs_t = pa_cs.tile([P, 512], dt.float32, tag="cs", name="cs_t")[:, :cw]
                nc.sync.dma_start(cs_t[:, :], cosT3[:, p, c0 : c0 + cw])
                sn_t = pa_cs.tile([P, 512], dt.float32, tag="sn", name="sn_t")[:, :cw]
                nc.sync.dma_start(sn_t[:, :], sinT3[:, p, c0 : c0 + cw])
                t0 = pa_tmp.tile([P, 512], dt.float32, tag="t0", name="t0")[:, :cw]
                nc.vector.tensor_copy(t0[:, :], pp[:, :])
                sw = pa_tmp.tile([P, 512], dt.float32, tag="sw", name="sw")[:, :cw]
                nc.sync.dma_start(sw[0::2, :], t0[1::2, :])
                nc.sync.dma_start(sw[1::2, :], t0[0::2, :])
                t1 = pa_tmp.tile([P, 512], dt.float32, tag="t1", name="t1")[:, :cw]
                nc.vector.tensor_tensor(t1[:, :], t0[:, :], cs_t[:, :], ALU.mult)
                t2 = pa_tmp.tile([P, 512], dt.float32, tag="t2", name="t2")[:, :cw]
                nc.vector.tensor_tensor(t2[:, :], sw[:, :], sn_t[:, :], ALU.mult)
                nc.vector.tensor_tensor(
                    dst[:, p, c0 : c0 + cw], t1[:, :], t2[:, :], ALU.add
                )

            # k projection + RoPE, v projection
            for cw, c0 in cidx:
                for p in range(2):
                    project_rope("k", kT, p, cw, c0)
                for kt in range(c0 // P, (c0 + cw) // P):
                    vp = ps_proj.tile([P, 512], dt.float32, tag="psm", name="vp")[
                        :, :CI
                    ]
                    for ko in range(KO):
                        nc.tensor.matmul(
                            vp[:, :],
                            xTr[:, ko, kt * P : (kt + 1) * P],
                            w_r["v"][:, ko, :],
                            start=(ko == 0),
                            stop=(ko == KO - 1),
                        )
                    nc.vector.tensor_scalar_mul(
                        v_sb[:, kt, :, 0:DH],
                        vp[:, :].rearrange("p (h d) -> p h d", h=HPC),
                        vm_sb[:, kt : kt + 1],
                    )
        # ---- phases B+C per q-chunk, q projection interleaved ----
        p_e = ctx.enter_context(tc.tile_pool(name="e", bufs=4))
        p_ctxT = ctx.enter_context(tc.tile_pool(name="ctxT", bufs=2))
        p_rv = ctx.enter_context(tc.tile_pool(name="rv", bufs=2))
        p_bc = ctx.enter_context(tc.tile_pool(name="bc", bufs=2))
        p_os = ctx.enter_context(tc.tile_pool(name="os", bufs=3))
        ps_sc = ctx.enter_context(tc.tile_pool(name="psS", bufs=2, space="PSUM"))
        ps_ctx = ctx.enter_context(tc.tile_pool(name="psC", bufs=2, space="PSUM"))
        
        kt_groups = [(k0, min(2, KT - k0)) for k0 in range(0, KT, 2)]

        for cw, c0 in cidx:
            for p in range(2):
                project_rope("q", qT, p, cw, c0)
            ctxT_sb = {}
            for hp in range(2):
                ctx_ps = []
                for hi in range(2):
                    cp = ps_ctx.tile([P, 512], dt.float32, tag="ctxp", name="cp")
                    ctx_ps.append(cp)
                for k0, g in kt_groups:
                    sp2 = {}
                    for hi in range(2):
                        sp2[hi] = ps_sc.tile(
                            [P, 2, 512], dt.float32, tag="sc", name="sp2"
                        )
                    # scores: alternate heads so PE row-groups 0-63/64-127
                    # run concurrently
                    for j in range(g):
                        kt = k0 + j
                        for hi in range(2):
                            nc.tensor.matmul(
                                sp2[hi][:, j, :cw],
                                kT[64 * hi : 64 * hi + 64, hp, kt * P : (kt + 1) * P],
                                qT[64 * hi : 64 * hi + 64, hp, c0 : c0 + cw],
                                start=True,
                                stop=True,
                            )
                    for hi in range(2):
                        h = 2 * hp + hi
                        e_sb = p_e.tile([P, 2, 512], dt.bfloat16, tag="e", name="e_sb")
                        nc.scalar.activation(
                            e_sb[:, :g, :cw], sp2[hi][:, :g, :cw], F.Exp
                        )
                        for j in range(g):
                            kt = k0 + j
                            nc.tensor.matmul(
                                ctx_ps[hi][0 : DH + 1, :cw],
                                v_sb[:, kt, h, :],
                                e_sb[:, j, :cw],
                                start=(kt == 0),
                                stop=(kt == KT - 1),
                            )
                # denominator rows -> one [2,cw] tile -> block ones-matmul bcast
                d2f = p_rv.tile([2, 512], dt.float32, tag="d2f", name="d2f")[:, :cw]
                nc.vector.tensor_copy(d2f[0:1, :], ctx_ps[0][DH : DH + 1, :cw])
                d1 = p_rv.tile([1, 512], dt.float32, tag="d1", name="d1")[:, :cw]
                nc.vector.tensor_copy(d1[:, :], ctx_ps[1][DH : DH + 1, :cw])
                nc.sync.dma_start(d2f[1:2, :], d1[:, :])
                d2r = p_rv.tile([2, 512], dt.float32r, tag="d2r", name="d2r")[:, :cw]
                nc.vector.tensor_copy(d2r[:, :], d2f[:, :])
                bp = ps_proj.tile([P, 512], dt.float32, tag="psm", name="bp")[:, :cw]
                nc.tensor.matmul(bp[:, :], ones2_r[:], d2r[:, :], start=True, stop=True)
                bc_sb = p_bc.tile([P, 512], dt.float32, tag="bc", name="bc_sb")[:, :cw]
                nc.vector.tensor_copy(bc_sb[:, :], bp[:, :])
                bc_r = p_bc.tile([P, 512], dt.float32, tag="bcr", name="bc_r")[:, :cw]
                nc.vector.reciprocal(bc_r[:, :], bc_sb[:, :])
                ct = p_ctxT.tile([P, 512], dt.bfloat16, tag=f"ctxT{hp}", name="ct")[:, :cw]
                nc.vector.tensor_tensor(
                    ct[0:64, :], ctx_ps[0][0:DH, :cw], bc_r[0:64, :], ALU.mult
                )
                nc.vector.tensor_tensor(
                    ct[64:128, :], ctx_ps[1][0:DH, :cw], bc_r[64:128, :], ALU.mult
                )
                ctxT_sb[hp] = ct

            for qs in range(cw // P):
                o_sb = p_os.tile([P, D_MODEL], dt.float32, tag="osb", name="o_sb")
                for ft in range(2):
                    op = ps_proj.tile([P, 512], dt.float32, tag="psm", name="op")
                    for hp in range(2):
                        nc.tensor.matmul(
                            op[:, :],
                            ctxT_sb[hp][:, qs * P : (qs + 1) * P],
                            wo_sb[:, hp, ft * 512 : (ft + 1) * 512],
                            start=(hp == 0),
                            stop=(hp == 1),
                        )
                    nc.vector.tensor_copy(o_sb[:, ft * 512 : (ft + 1) * 512], op[:])
                nc.sync.dma_start(out[c0 + qs * P : c0 + (qs + 1) * P, :], o_sb[:])

    return nc


_cache = {}


def _get_program(LK):
    if LK not in _cache:
        _cache[LK] = _build(LK)
    return _cache[LK]


def kernel(**inputs) -> np.ndarray:
    import ml_dtypes

    x = np.asarray(inputs["x"], dtype=np.float32)
    rope_cos = np.asarray(inputs["rope_cos"], dtype=np.float32)
    rope_sin = np.asarray(inputs["rope_sin"], dtype=np.float32)
    L = np.asarray(inputs["input_lengths"]).astype(np.int64)
    Wq = np.asarray(inputs["Wq"], dtype=np.float32)
    Wk = np.asarray(inputs["Wk"], dtype=np.float32)
    Wv = np.asarray(inputs["Wv"], dtype=np.float32)
    Wo = np.asarray(inputs["Wo"], dtype=np.float32)
    bq = np.asarray(inputs["bq"], dtype=np.float32)
    bk = np.asarray(inputs["bk"], dtype=np.float32)
    bv = np.asarray(inputs["bv"], dtype=np.float32)
    bo = np.asarray(inputs["bo"], dtype=np.float32)

    Bn, Nn, Dn = x.shape
    assert (Bn, Nn, Dn) == (B, N, D_MODEL)
    assert not (np.any(bq) or np.any(bk) or np.any(bv)), (
        "nonzero qkv biases not supported by this kernel build"
    )

    Lmax = int(L.max())
    LK = max(((Lmax + P - 1) // P) * P, 256)
    KT = LK // P

    nc = _get_program(LK)

    sign = np.where(np.arange(CI) % 2 == 0, -1.0, 1.0).astype(np.float32)
    ones2 = np.zeros((2, P), np.float32)
    ones2[0, 0:64] = 1.0
    ones2[1, 64:128] = 1.0
    karr = np.arange(P)[:, None] + P * np.arange(KT)[None, :]

    in_maps = []
    for c in range(N_CORES):
        b, g = divmod(c, HPC)
        cols = slice(CI * g, CI * (g + 1))
        in_maps.append(
            {
                "xT": np.ascontiguousarray(x[b, :LK, :].T),
                "wq": np.ascontiguousarray(Wq[:, cols]),
                "wk": np.ascontiguousarray(Wk[:, cols]),
                "wv": np.ascontiguousarray(Wv[:, cols]),
                "wo": np.ascontiguousarray(Wo[cols, :]).astype(ml_dtypes.bfloat16),
                "cosT": np.ascontiguousarray(rope_cos[b, :LK, cols].T),
                "sinT": np.ascontiguousarray(rope_sin[b, :LK, cols].T * sign[:, None]),
                "vmask": (karr < L[b]).astype(np.float32),
                "ones2": ones2,
            }
        )

    res = run_bass_kernel_spmd(nc, in_maps, core_ids=list(range(N_CORES)))

    out = np.zeros((B, N, D_MODEL), np.float32)
    for c in range(N_CORES):
        b = c // HPC
        out[b, :LK, :] += res.results[c]["out"]
    for b in range(B):
        out[b, L[b] :, :] = 0.0
    out += bo[None, None, :]
    out *= (np.arange(N)[None, :] < L[:, None])[:, :, None].astype(np.float32)
    return out


# revision 19
# speedup vs baseline: 1.1937x; 1.1937x over previous
"""Multi-head attention (RoPE + length masking) on 8 Trainium2 NeuronCores.

Sharding: core c handles batch b = c // 4 and heads [4*(c%4), 4*(c%4)+4).
Each core computes q/k/v projections for its 256 inner dims, RoPE, per-head
attention with length masking, and a row-parallel slice of the output
projection; the host sums the 4 partial outputs per batch (all-reduce) and
applies the final key-mask zeroing.

Precision: float32r (reduced-mantissa fp32, full PE rate) for projections
and scores; bf16 for exp(scores), v, and the output projection. Measured
end-to-end relative error ~2e-3.
"""

import numpy as np
from contextlib import ExitStack

import bass_rust as _br
import concourse.bass as bass
import concourse.tile as tile
import concourse.mybir as mybir
from concourse.bass_utils import run_bass_kernel_spmd

P = 128
B, N, D_MODEL, H, DH = 2, 2048, 1024, 16, 64
N_CORES = 8
HPC = 4            # heads per core
CI = HPC * DH      # per-core inner dim (256)
KO = D_MODEL // P  # 8 contraction tiles

_counter = [0]


def _split_excess_waits(nc, max_normal=1, max_evsem=1):
    """Walrus in this toolchain rejects >1 semaphore wait per instruction.
    Hoist excess waits onto EventSemaphore carriers inserted right before
    the offending instruction in the same engine's program order."""
    for fn in nc.m.functions:
        for bb in fn.blocks:
            insts = bb.instructions
            out = []
            changed = False
            for inst in insts:
                si = inst.sync_info
                waits = list(si.on_wait or []) if si is not None else []
                cap = (
                    max_evsem
                    if isinstance(inst, mybir.InstEventSemaphore)
                    else max_normal
                )
                if len(waits) > cap:
                    extra, keep = waits[:-cap], waits[-cap:]
                    si.on_wait = keep
                    for i in range(0, len(extra), max_evsem):
                        _counter[0] += 1
                        ev = mybir.InstEventSemaphore(
                            name=f"I-waitsplit-{_counter[0]}",
                            engine=inst.engine,
                            sync_info=_br.SyncInfo(
                                on_wait=extra[i : i + max_evsem], on_update=[]
                            ),
                            ins=[],
                            outs=[],
                        )
                        out.append(ev)
                    changed = True
                out.append(inst)
            if changed:
                bb.instructions = out


class _TileContextFixed(tile.TileContext):
    def _drain_and_barrier(self, tick_clock, wait_clock):
        from concourse.tile import ScopedClock

        nc = self.nc
        drain_inst = nc.sync.drain()
        wait_clock.add_sem_waits(
            drain_inst.ins, ScopedClock({None: tick_clock.global_clock})
        )
        nc.all_engine_barrier()
        assert self.sems is not None
        popped = nc._tile_sem_poison_stack.pop()
        assert popped is self._sem_poison
        nc.clear_and_free_semaphores(list(self.sems.allocated().values()))
        nc.all_engine_barrier()
        _split_excess_waits(nc)


def _make_chunks(LK):
    """Split LK (multiple of 128) into free-dim chunks, preferring 512 and
    keeping every chunk >= 256 when possible (f32r full-rate needs >= 256)."""
    chunks = [512] * (LK // 512)
    rem = LK % 512
    if rem:
        if rem >= 256 or not chunks:
            chunks.append(rem)
        else:
            # e.g. rem=128 -> replace one 512 with 384 + 256
            chunks[-1] = 512 - (256 - rem)
            chunks.append(256)
    return chunks


def _build(LK):
    """Build the single-core Bass program (same program on all 8 cores)."""
    dt = mybir.dt
    KT = LK // P
    chunks = _make_chunks(LK)
    starts = np.cumsum([0] + chunks[:-1]).tolist()
    cidx = list(zip(chunks, starts))

    nc = bass.Bass(trn_type="TRN2")

    xT = nc.dram_tensor("xT", [D_MODEL, LK], dt.float32, kind="ExternalInput")
    wq = nc.dram_tensor("wq", [D_MODEL, CI], dt.float32, kind="ExternalInput")
    wk = nc.dram_tensor("wk", [D_MODEL, CI], dt.float32, kind="ExternalInput")
    wv = nc.dram_tensor("wv", [D_MODEL, CI], dt.float32, kind="ExternalInput")
    wo = nc.dram_tensor("wo", [CI, D_MODEL], dt.bfloat16, kind="ExternalInput")
    cosT = nc.dram_tensor("cosT", [CI, LK], dt.float32, kind="ExternalInput")
    sinT = nc.dram_tensor("sinT", [CI, LK], dt.float32, kind="ExternalInput")
    vmask = nc.dram_tensor("vmask", [P, KT], dt.float32, kind="ExternalInput")
    ones2 = nc.dram_tensor("ones2", [2, P], dt.float32, kind="ExternalInput")
    out = nc.dram_tensor("out", [LK, D_MODEL], dt.float32, kind="ExternalOutput")

    F = mybir.ActivationFunctionType
    ALU = mybir.AluOpType

    with _TileContextFixed(nc) as tc, ExitStack() as ctx:
        # ---- long-lived pools ----
        p_qk = ctx.enter_context(tc.tile_pool(name="qk", bufs=1))
        p_v = ctx.enter_context(tc.tile_pool(name="v", bufs=1))
        p_const = ctx.enter_context(tc.tile_pool(name="const", bufs=1))

        qT = p_qk.tile([P, 2, LK], dt.float32r)
        kT = p_qk.tile([P, 2, LK], dt.float32r)
        v_sb = p_v.tile([P, KT, HPC, DH + 1], dt.bfloat16)
        wo_sb = p_const.tile([P, 2, D_MODEL], dt.bfloat16)
        vm_sb = p_const.tile([P, KT], dt.float32)
        ones2_r = p_const.tile([2, P], dt.float32r)

        nc.sync.dma_start(vm_sb[:], vmask[:])
        nc.sync.dma_start(wo_sb[:], wo.rearrange("(po pi) f -> pi po f", pi=P))
        ones2_f = p_const.tile([2, P], dt.float32)
        nc.sync.dma_start(ones2_f[:], ones2[:])
        nc.vector.tensor_copy(ones2_r[:], ones2_f[:])
        # ones column of v_aug carries the k-mask (0 rows beyond length)
        nc.vector.tensor_copy(
            v_sb[:, :, :, DH],
            vm_sb[:, :, None].to_broadcast((P, KT, HPC)),
        )

        # ---- phase A: load x/W chunk-wise; project k, v; q is emitted
        #      per-chunk inside the attention loop for overlap ----
        if True:
            actx = ctx
            pa_w = actx.enter_context(tc.tile_pool(name="wstage", bufs=1))
            pa_x = actx.enter_context(tc.tile_pool(name="xtr", bufs=1))
            pa_stage = actx.enter_context(tc.tile_pool(name="stage", bufs=2))
            pa_tmp = actx.enter_context(tc.tile_pool(name="ropetmp", bufs=2))
            pa_cs = actx.enter_context(tc.tile_pool(name="cschunk", bufs=2))
            ps_proj = actx.enter_context(
                tc.tile_pool(name="psQ", bufs=1, space="PSUM")
            )
            ps_mix = actx.enter_context(
                tc.tile_pool(name="psM", bufs=1, space="PSUM")
            )

            cosT3 = cosT.rearrange("(po pi) s -> pi po s", pi=P)
            sinT3 = sinT.rearrange("(po pi) s -> pi po s", pi=P)
            w_r = {}
            wstg = {}
            for name, wdram in (("k", wk), ("v", wv), ("q", wq)):
                wst = pa_stage.tile([P, KO, CI], dt.float32, tag="wst", name="wst")
                nc.sync.dma_start(
                    wst[:], wdram.rearrange("(ko ki) c -> ki ko c", ki=P)
                )
                wstg[name] = wst
            for name in ("k", "v", "q"):
                wr = pa_w.tile([P, KO, CI], dt.float32r, tag=f"w{name}", name="wr")
                if name == "k":
                    nc.vector.tensor_copy(wr[:], wstg[name][:])
                else:
                    nc.gpsimd.tensor_copy(wr[:], wstg[name][:])
                w_r[name] = wr

            xTr = pa_x.tile([P, KO, LK], dt.float32r)
            xT3 = xT.rearrange("(ko ki) s -> ki ko s", ki=P)
            for cw, c0 in cidx:
                for ko in range(KO):
                    xst = pa_stage.tile(
                        [P, 512], dt.float32, tag="xst", name="xst"
                    )[:, :cw]
                    nc.sync.dma_start(xst[:, :], xT3[:, ko, c0 : c0 + cw])
                    eng = nc.vector if ko % 2 == 0 else nc.gpsimd
                    eng.tensor_copy(xTr[:, ko, c0 : c0 + cw], xst[:, :])

            def project_rope(name, dst, p, cw, c0):
                pp = ps_proj.tile([P, 512], dt.float32, tag="psm", name="pp")[:, :cw]
                for ko in range(KO):
                    nc.tensor.matmul(
                        pp[:, :],
                        w_r[name][:, ko, p * P : (p + 1) * P],
                        xTr[:, ko, c0 : c0 + cw],
                        start=(ko == 0),
                        stop=(ko == KO - 1),
                    )
                cs_t = pa_cs.tile([P, 512], dt.float32, tag="cs", name="cs_t")[:, :cw]
                nc.sync.dma_start(cs_t[:, :], cosT3[:, p, c0 : c0 + cw])
                sn_t = pa_cs.tile([P, 512], dt.float32, tag="sn", name="sn_t")[:, :cw]
                nc.sync.dma_start(sn_t[:, :], sinT3[:, p, c0 : c0 + cw])
                t0 = pa_tmp.tile([P, 512], dt.float32, tag="t0", name="t0")[:, :cw]
                nc.vector.tensor_copy(t0[:, :], pp[:, :])
                sw = pa_tmp.tile([P, 512], dt.float32, tag="sw", name="sw")[:, :cw]
                nc.sync.dma_start(sw[0::2, :], t0[1::2, :])
                nc.sync.dma_start(sw[1::2, :], t0[0::2, :])
                t1 = pa_tmp.tile([P, 512], dt.float32, tag="t1", name="t1")[:, :cw]
                nc.vector.tensor_tensor(t1[:, :], t0[:, :], cs_t[:, :], ALU.mult)
                t2 = pa_tmp.tile([P, 512], dt.float32, tag="t2", name="t2")[:, :cw]
                nc.vector.tensor_tensor(t2[:, :], sw[:, :], sn_t[:, :], ALU.mult)
                nc.vector.tensor_tensor(
                    dst[:, p, c0 : c0 + cw], t1[:, :], t2[:, :], ALU.add
                )

            # k projection + RoPE, v projection
            for cw, c0 in cidx:
                for p in range(2):
                    project_rope("k", kT, p, cw, c0)
                for kt in range(c0 // P, (c0 + cw) // P):
                    vp = ps_proj.tile([P, 512], dt.float32, tag="psm", name="vp")[
                        :, :CI
                    ]
                    for ko in range(KO):
                        nc.tensor.matmul(
                            vp[:, :],
                            xTr[:, ko, kt * P : (kt + 1) * P],
                            w_r["v"][:, ko, :],
                            start=(ko == 0),
                            stop=(ko == KO - 1),
                        )
                    nc.vector.tensor_scalar_mul(
                        v_sb[:, kt, :, 0:DH],
                        vp[:, :].rearrange("p (h d) -> p h d", h=HPC),
                        vm_sb[:, kt : kt + 1],
                    )
        # ---- phases B+C per q-chunk, q projection interleaved ----
        p_e = ctx.enter_context(tc.tile_pool(name="e", bufs=4))
        p_ctxT = ctx.enter_context(tc.tile_pool(name="ctxT", bufs=2))
        p_rv = ctx.enter_context(tc.tile_pool(name="rv", bufs=2))
        p_bc = ctx.enter_context(tc.tile_pool(name="bc", bufs=2))
        p_os = ctx.enter_context(tc.tile_pool(name="os", bufs=3))
        ps_sc = ctx.enter_context(tc.tile_pool(name="psS", bufs=2, space="PSUM"))
        ps_ctx = ctx.enter_context(tc.tile_pool(name="psC", bufs=2, space="PSUM"))
        
        kt_groups = [(k0, min(2, KT - k0)) for k0 in range(0, KT, 2)]

        for cw, c0 in cidx:
            for p in range(2):
                project_rope("q", qT, p, cw, c0)
            ctxT_sb = {}
            for hp in range(2):
                ctx_ps = []
                for hi in range(2):
                    cp = ps_ctx.tile([P, 512], dt.float32, tag="ctxp", name="cp")
                    ctx_ps.append(cp)
                for k0, g in kt_groups:
                    sp2 = {}
                    for hi in range(2):
                        sp2[hi] = ps_sc.tile(
                            [P, 2, 512], dt.float32, tag="sc", name="sp2"
                        )
                    # scores: alternate heads so PE row-groups 0-63/64-127
                    # run concurrently
                    for j in range(g):
                        kt = k0 + j
                        for hi in range(2):
                            nc.tensor.matmul(
                                sp2[hi][:, j, :cw],
                                kT[64 * hi : 64 * hi + 64, hp, kt * P : (kt + 1) * P],
                                qT[64 * hi : 64 * hi + 64, hp, c0 : c0 + cw],
                                start=True,
                                stop=True,
                            )
                    for hi in range(2):
                        h = 2 * hp + hi
                        e_sb = p_e.tile([P, 2, 512], dt.bfloat16, tag="e", name="e_sb")
                        nc.scalar.activation(
                            e_sb[:, :g, :cw], sp2[hi][:, :g, :cw], F.Exp
                        )
                        for j in range(g):
                            kt = k0 + j
                            nc.tensor.matmul(
                                ctx_ps[hi][0 : DH + 1, :cw],
                                v_sb[:, kt, h, :],
                                e_sb[:, j, :cw],
                                start=(kt == 0),
                                stop=(kt == KT - 1),
                            )
                # denominator rows -> one [2,cw] tile -> block ones-matmul bcast
                d2f = p_rv.tile([2, 512], dt.float32, tag="d2f", name="d2f")[:, :cw]
                nc.vector.tensor_copy(d2f[0:1, :], ctx_ps[0][DH : DH + 1, :cw])
                d1 = p_rv.tile([1, 512], dt.float32, tag="d1", name="d1")[:, :cw]
                nc.vector.tensor_copy(d1[:, :], ctx_ps[1][DH : DH + 1, :cw])
                nc.sync.dma_start(d2f[1:2, :], d1[:, :])
                d2r = p_rv.tile([2, 512], dt.float32r, tag="d2r", name="d2r")[:, :cw]
                nc.vector.tensor_copy(d2r[:, :], d2f[:, :])
                bp = ps_mix.tile([P, 512], dt.float32, tag="psm", name="bp")[:, :cw]
                nc.tensor.matmul(bp[:, :], ones2_r[:], d2r[:, :], start=True, stop=True)
                bc_sb = p_bc.tile([P, 512], dt.float32, tag="bc", name="bc_sb")[:, :cw]
                nc.vector.tensor_copy(bc_sb[:, :], bp[:, :])
                bc_r = p_bc.tile([P, 512], dt.float32, tag="bcr", name="bc_r")[:, :cw]
                nc.vector.reciprocal(bc_r[:, :], bc_sb[:, :])
                ct = p_ctxT.tile([P, 512], dt.bfloat16, tag=f"ctxT{hp}", name="ct")[:, :cw]
                nc.vector.tensor_tensor(
                    ct[0:64, :], ctx_ps[0][0:DH, :cw], bc_r[0:64, :], ALU.mult
                )
                nc.vector.tensor_tensor(
                    ct[64:128, :], ctx_ps[1][0:DH, :cw], bc_r[64:128, :], ALU.mult
                )
                ctxT_sb[hp] = ct

            for qs in range(cw // P):
                o_sb = p_os.tile([P, D_MODEL], dt.float32, tag="osb", name="o_sb")
                for ft in range(2):
                    op = ps_mix.tile([P, 512], dt.float32, tag="psm", name="op")
                    for hp in range(2):
                        nc.tensor.matmul(
                            op[:, :],
                            ctxT_sb[hp][:, qs * P : (qs + 1) * P],
                            wo_sb[:, hp, ft * 512 : (ft + 1) * 512],
                            start=(hp == 0),
                            stop=(hp == 1),
                        )
                    nc.vector.tensor_copy(o_sb[:, ft * 512 : (ft + 1) * 512], op[:])
                nc.sync.dma_start(out[c0 + qs * P : c0 + (qs + 1) * P, :], o_sb[:])

    return nc


_cache = {}


def _get_program(LK):
    if LK not in _cache:
        _cache[LK] = _build(LK)
    return _cache[LK]


def kernel(**inputs) -> np.ndarray:
    import ml_dtypes

    x = np.asarray(inputs["x"], dtype=np.float32)
    rope_cos = np.asarray(inputs["rope_cos"], dtype=np.float32)
    rope_sin = np.asarray(inputs["rope_sin"], dtype=np.float32)
    L = np.asarray(inputs["input_lengths"]).astype(np.int64)
    Wq = np.asarray(inputs["Wq"], dtype=np.float32)
    Wk = np.asarray(inputs["Wk"], dtype=np.float32)
    Wv = np.asarray(inputs["Wv"], dtype=np.float32)
    Wo = np.asarray(inputs["Wo"], dtype=np.float32)
    bq = np.asarray(inputs["bq"], dtype=np.float32)
    bk = np.asarray(inputs["bk"], dtype=np.float32)
    bv = np.asarray(inputs["bv"], dtype=np.float32)
    bo = np.asarray(inputs["bo"], dtype=np.float32)

    Bn, Nn, Dn = x.shape
    assert (Bn, Nn, Dn) == (B, N, D_MODEL)
    assert not (np.any(bq) or np.any(bk) or np.any(bv)), (
        "nonzero qkv biases not supported by this kernel build"
    )

    Lmax = int(L.max())
    LK = max(((Lmax + P - 1) // P) * P, 256)
    KT = LK // P

    nc = _get_program(LK)

    sign = np.where(np.arange(CI) % 2 == 0, -1.0, 1.0).astype(np.float32)
    ones2 = np.zeros((2, P), np.float32)
    ones2[0, 0:64] = 1.0
    ones2[1, 64:128] = 1.0
    karr = np.arange(P)[:, None] + P * np.arange(KT)[None, :]

    in_maps = []
    for c in range(N_CORES):
        b, g = divmod(c, HPC)
        cols = slice(CI * g, CI * (g + 1))
        in_maps.append(
            {
                "xT": np.ascontiguousarray(x[b, :LK, :].T),
                "wq": np.ascontiguousarray(Wq[:, cols]),
                "wk": np.ascontiguousarray(Wk[:, cols]),
                "wv": np.ascontiguousarray(Wv[:, cols]),
                "wo": np.ascontiguousarray(Wo[cols, :]).astype(ml_dtypes.bfloat16),
                "cosT": np.ascontiguousarray(rope_cos[b, :LK, cols].T),
                "sinT": np.ascontiguousarray(rope_sin[b, :LK, cols].T * sign[:, None]),
                "vmask": (karr < L[b]).astype(np.float32),
                "ones2": ones2,
            }
        )

    res = run_bass_kernel_spmd(nc, in_maps, core_ids=list(range(N_CORES)))

    out = np.zeros((B, N, D_MODEL), np.float32)
    for c in range(N_CORES):
        b = c // HPC
        out[b, :LK, :] += res.results[c]["out"]
    for b in range(B):
        out[b, L[b] :, :] = 0.0
    out += bo[None, None, :]
    out *= (np.arange(N)[None, :] < L[:, None])[:, :, None].astype(np.float32)
    return out


# revision 21
# speedup vs baseline: 1.3260x; 1.1108x over previous
"""Multi-head attention (RoPE + length masking) on 8 Trainium2 NeuronCores.

Sharding: core c handles batch b = c // 4 and heads [4*(c%4), 4*(c%4)+4).
Each core computes q/k/v projections for its 256 inner dims, RoPE, per-head
attention with length masking, and a row-parallel slice of the output
projection; the host sums the 4 partial outputs per batch (all-reduce) and
applies the final key-mask zeroing.

Precision: float32r (reduced-mantissa fp32, full PE rate) for projections
and scores; bf16 for exp(scores), v, and the output projection. Measured
end-to-end relative error ~2e-3.
"""

import numpy as np
from contextlib import ExitStack

import bass_rust as _br
import concourse.bass as bass
import concourse.tile as tile
import concourse.mybir as mybir
from concourse.bass_utils import run_bass_kernel_spmd

P = 128
B, N, D_MODEL, H, DH = 2, 2048, 1024, 16, 64
N_CORES = 8
HPC = 4            # heads per core
CI = HPC * DH      # per-core inner dim (256)
KO = D_MODEL // P  # 8 contraction tiles

_counter = [0]


def _split_excess_waits(nc, max_normal=1, max_evsem=1):
    """Walrus in this toolchain rejects >1 semaphore wait per instruction.
    Hoist excess waits onto EventSemaphore carriers inserted right before
    the offending instruction in the same engine's program order."""
    for fn in nc.m.functions:
        for bb in fn.blocks:
            insts = bb.instructions
            out = []
            changed = False
            for inst in insts:
                si = inst.sync_info
                waits = list(si.on_wait or []) if si is not None else []
                cap = (
                    max_evsem
                    if isinstance(inst, mybir.InstEventSemaphore)
                    else max_normal
                )
                if len(waits) > cap:
                    extra, keep = waits[:-cap], waits[-cap:]
                    si.on_wait = keep
                    for i in range(0, len(extra), max_evsem):
                        _counter[0] += 1
                        ev = mybir.InstEventSemaphore(
                            name=f"I-waitsplit-{_counter[0]}",
                            engine=inst.engine,
                            sync_info=_br.SyncInfo(
                                on_wait=extra[i : i + max_evsem], on_update=[]
                            ),
                            ins=[],
                            outs=[],
                        )
                        out.append(ev)
                    changed = True
                out.append(inst)
            if changed:
                bb.instructions = out


class _TileContextFixed(tile.TileContext):
    def _drain_and_barrier(self, tick_clock, wait_clock):
        from concourse.tile import ScopedClock

        nc = self.nc
        drain_inst = nc.sync.drain()
        wait_clock.add_sem_waits(
            drain_inst.ins, ScopedClock({None: tick_clock.global_clock})
        )
        nc.all_engine_barrier()
        assert self.sems is not None
        popped = nc._tile_sem_poison_stack.pop()
        assert popped is self._sem_poison
        nc.clear_and_free_semaphores(list(self.sems.allocated().values()))
        nc.all_engine_barrier()
        _split_excess_waits(nc)


def _make_chunks(LK):
    """Split LK (multiple of 128) into free-dim chunks, preferring 512 and
    keeping every chunk >= 256 when possible (f32r full-rate needs >= 256)."""
    chunks = [512] * (LK // 512)
    rem = LK % 512
    if rem:
        if rem >= 256 or not chunks:
            chunks.append(rem)
        else:
            # e.g. rem=128 -> replace one 512 with 384 + 256
            chunks[-1] = 512 - (256 - rem)
            chunks.append(256)
    return chunks


def _build(LK):
    """Build the single-core Bass program (same program on all 8 cores)."""
    dt = mybir.dt
    KT = LK // P
    chunks = _make_chunks(LK)
    starts = np.cumsum([0] + chunks[:-1]).tolist()
    cidx = list(zip(chunks, starts))

    nc = bass.Bass(trn_type="TRN2")

    xT = nc.dram_tensor("xT", [D_MODEL, LK], dt.float32, kind="ExternalInput")
    wq = nc.dram_tensor("wq", [D_MODEL, CI], dt.float32, kind="ExternalInput")
    wk = nc.dram_tensor("wk", [D_MODEL, CI], dt.float32, kind="ExternalInput")
    wv = nc.dram_tensor("wv", [D_MODEL, CI], dt.float32, kind="ExternalInput")
    wo = nc.dram_tensor("wo", [CI, D_MODEL], dt.bfloat16, kind="ExternalInput")
    cosT = nc.dram_tensor("cosT", [CI, LK], dt.float32, kind="ExternalInput")
    sinT = nc.dram_tensor("sinT", [CI, LK], dt.float32, kind="ExternalInput")
    vmask = nc.dram_tensor("vmask", [P, KT], dt.float32, kind="ExternalInput")
    ones2 = nc.dram_tensor("ones2", [2, P], dt.float32, kind="ExternalInput")
    out = nc.dram_tensor("out", [LK, D_MODEL], dt.float32, kind="ExternalOutput")

    F = mybir.ActivationFunctionType
    ALU = mybir.AluOpType

    with _TileContextFixed(nc) as tc, ExitStack() as ctx:
        # ---- long-lived pools ----
        p_qk = ctx.enter_context(tc.tile_pool(name="qk", bufs=1))
        p_v = ctx.enter_context(tc.tile_pool(name="v", bufs=1))
        p_const = ctx.enter_context(tc.tile_pool(name="const", bufs=1))

        qT = p_qk.tile([P, 2, LK], dt.float32r)
        kT = p_qk.tile([P, 2, LK], dt.float32r)
        v_sb = p_v.tile([P, KT, HPC, DH + 1], dt.bfloat16)
        wo_sb = p_const.tile([P, 2, D_MODEL], dt.bfloat16)
        vm_sb = p_const.tile([P, KT], dt.float32)
        ones2_r = p_const.tile([2, P], dt.float32r)

        nc.sync.dma_start(vm_sb[:], vmask[:])
        nc.sync.dma_start(wo_sb[:], wo.rearrange("(po pi) f -> pi po f", pi=P))
        ones2_f = p_const.tile([2, P], dt.float32)
        nc.sync.dma_start(ones2_f[:], ones2[:])
        nc.vector.tensor_copy(ones2_r[:], ones2_f[:])
        # ones column of v_aug carries the k-mask (0 rows beyond length)
        nc.vector.tensor_copy(
            v_sb[:, :, :, DH],
            vm_sb[:, :, None].to_broadcast((P, KT, HPC)),
        )

        # ---- phase A (scoped pools): load chunk-wise, project k, v, q ----
        with ExitStack() as actx:
            pa_w = actx.enter_context(tc.tile_pool(name="wstage", bufs=1))
            pa_x = actx.enter_context(tc.tile_pool(name="xtr", bufs=1))
            pa_stage = actx.enter_context(tc.tile_pool(name="stage", bufs=4))
            pa_tmp = actx.enter_context(tc.tile_pool(name="ropetmp", bufs=3))
            pa_cs = actx.enter_context(tc.tile_pool(name="cschunk", bufs=3))
            ps_proj = actx.enter_context(
                tc.tile_pool(name="psA", bufs=2, space="PSUM")
            )

            cosT3 = cosT.rearrange("(po pi) s -> pi po s", pi=P)
            sinT3 = sinT.rearrange("(po pi) s -> pi po s", pi=P)
            w_r = {}
            # k weights first: they gate the first projection matmuls
            wst_k = pa_stage.tile([P, KO, CI], dt.float32, tag="wst", name="wst_k")
            nc.sync.dma_start(wst_k[:], wk.rearrange("(ko ki) c -> ki ko c", ki=P))
            wr_k = pa_w.tile([P, KO, CI], dt.float32r, tag="wkr", name="wr_k")
            nc.vector.tensor_copy(wr_k[:], wst_k[:])
            w_r["k"] = wr_k

            xTr = pa_x.tile([P, KO, LK], dt.float32r)
            xT3 = xT.rearrange("(ko ki) s -> ki ko s", ki=P)
            for cw, c0 in cidx:
                for ko in range(KO):
                    xst = pa_stage.tile(
                        [P, 512], dt.float32, tag="xst", name="xst"
                    )[:, :cw]
                    nc.sync.dma_start(xst[:, :], xT3[:, ko, c0 : c0 + cw])
                    eng = nc.vector if ko % 2 == 0 else nc.gpsimd
                    eng.tensor_copy(xTr[:, ko, c0 : c0 + cw], xst[:, :])

            for name, wdram in (("v", wv), ("q", wq)):
                wst = pa_stage.tile([P, KO, CI], dt.float32, tag="wst", name="wst")
                nc.sync.dma_start(
                    wst[:], wdram.rearrange("(ko ki) c -> ki ko c", ki=P)
                )
                wr = pa_w.tile([P, KO, CI], dt.float32r, tag=f"w{name}", name="wr")
                nc.gpsimd.tensor_copy(wr[:], wst[:])
                w_r[name] = wr

            def project_rope(name, dst, p, cw, c0):
                pp = ps_proj.tile([P, 512], dt.float32, tag="psA", name="pp")[:, :cw]
                for ko in range(KO):
                    nc.tensor.matmul(
                        pp[:, :],
                        w_r[name][:, ko, p * P : (p + 1) * P],
                        xTr[:, ko, c0 : c0 + cw],
                        start=(ko == 0),
                        stop=(ko == KO - 1),
                    )
                cs_t = pa_cs.tile([P, 512], dt.float32, tag="cs", name="cs_t")[:, :cw]
                nc.sync.dma_start(cs_t[:, :], cosT3[:, p, c0 : c0 + cw])
                sn_t = pa_cs.tile([P, 512], dt.float32, tag="sn", name="sn_t")[:, :cw]
                nc.sync.dma_start(sn_t[:, :], sinT3[:, p, c0 : c0 + cw])
                t0 = pa_tmp.tile([P, 512], dt.float32, tag="t0", name="t0")[:, :cw]
                nc.vector.tensor_copy(t0[:, :], pp[:, :])
                sw = pa_tmp.tile([P, 512], dt.float32, tag="sw", name="sw")[:, :cw]
                nc.sync.dma_start(sw[0::2, :], t0[1::2, :])
                nc.sync.dma_start(sw[1::2, :], t0[0::2, :])
                t1 = pa_tmp.tile([P, 512], dt.float32, tag="t1", name="t1")[:, :cw]
                nc.vector.tensor_tensor(t1[:, :], t0[:, :], cs_t[:, :], ALU.mult)
                t2 = pa_tmp.tile([P, 512], dt.float32, tag="t2", name="t2")[:, :cw]
                nc.vector.tensor_tensor(t2[:, :], sw[:, :], sn_t[:, :], ALU.mult)
                nc.vector.tensor_tensor(
                    dst[:, p, c0 : c0 + cw], t1[:, :], t2[:, :], ALU.add
                )

            for cw, c0 in cidx:
                for p in range(2):
                    project_rope("k", kT, p, cw, c0)
                for kt in range(c0 // P, (c0 + cw) // P):
                    vp = ps_proj.tile([P, 512], dt.float32, tag="psA", name="vp")[
                        :, :CI
                    ]
                    for ko in range(KO):
                        nc.tensor.matmul(
                            vp[:, :],
                            xTr[:, ko, kt * P : (kt + 1) * P],
                            w_r["v"][:, ko, :],
                            start=(ko == 0),
                            stop=(ko == KO - 1),
                        )
                    nc.vector.tensor_scalar_mul(
                        v_sb[:, kt, :, 0:DH],
                        vp[:, :].rearrange("p (h d) -> p h d", h=HPC),
                        vm_sb[:, kt : kt + 1],
                    )
            for cw, c0 in cidx:
                for p in range(2):
                    project_rope("q", qT, p, cw, c0)

        # ---- phases B+C per q-chunk ----
        p_e = ctx.enter_context(tc.tile_pool(name="e", bufs=4))
        p_ctxT = ctx.enter_context(tc.tile_pool(name="ctxT", bufs=2))
        p_rv = ctx.enter_context(tc.tile_pool(name="rv", bufs=2))
        p_bc = ctx.enter_context(tc.tile_pool(name="bc", bufs=2))
        p_os = ctx.enter_context(tc.tile_pool(name="os", bufs=3))
        ps_sc = ctx.enter_context(tc.tile_pool(name="psS", bufs=2, space="PSUM"))
        ps_ctx = ctx.enter_context(tc.tile_pool(name="psC", bufs=3, space="PSUM"))
        ps_mix = ctx.enter_context(tc.tile_pool(name="psM", bufs=1, space="PSUM"))

        kt_groups = [(k0, min(2, KT - k0)) for k0 in range(0, KT, 2)]

        for cw, c0 in cidx:
            ctxT_sb = {}
            for hp in range(2):
                ctx_ps = []
                for hi in range(2):
                    cp = ps_ctx.tile([P, 512], dt.float32, tag="ctxp", name="cp")
                    ctx_ps.append(cp)
                for k0, g in kt_groups:
                    sp2 = {}
                    for hi in range(2):
                        sp2[hi] = ps_sc.tile(
                            [P, 2, 512], dt.float32, tag="sc", name="sp2"
                        )
                    # scores: alternate heads so PE row-groups 0-63/64-127
                    # run concurrently
                    for j in range(g):
                        kt = k0 + j
                        for hi in range(2):
                            nc.tensor.matmul(
                                sp2[hi][:, j, :cw],
                                kT[64 * hi : 64 * hi + 64, hp, kt * P : (kt + 1) * P],
                                qT[64 * hi : 64 * hi + 64, hp, c0 : c0 + cw],
                                start=True,
                                stop=True,
                            )
                    for hi in range(2):
                        h = 2 * hp + hi
                        e_sb = p_e.tile([P, 2, 512], dt.bfloat16, tag="e", name="e_sb")
                        nc.scalar.activation(
                            e_sb[:, :g, :cw], sp2[hi][:, :g, :cw], F.Exp
                        )
                        for j in range(g):
                            kt = k0 + j
                            nc.tensor.matmul(
                                ctx_ps[hi][0 : DH + 1, :cw],
                                v_sb[:, kt, h, :],
                                e_sb[:, j, :cw],
                                start=(kt == 0),
                                stop=(kt == KT - 1),
                            )
                # denominator rows -> one [2,cw] tile -> block ones-matmul bcast
                d2f = p_rv.tile([2, 512], dt.float32, tag="d2f", name="d2f")[:, :cw]
                nc.vector.tensor_copy(d2f[0:1, :], ctx_ps[0][DH : DH + 1, :cw])
                d1 = p_rv.tile([1, 512], dt.float32, tag="d1", name="d1")[:, :cw]
                nc.vector.tensor_copy(d1[:, :], ctx_ps[1][DH : DH + 1, :cw])
                nc.sync.dma_start(d2f[1:2, :], d1[:, :])
                d2r = p_rv.tile([2, 512], dt.float32r, tag="d2r", name="d2r")[:, :cw]
                nc.vector.tensor_copy(d2r[:, :], d2f[:, :])
                bp = ps_mix.tile([P, 512], dt.float32, tag="psm", name="bp")[:, :cw]
                nc.tensor.matmul(bp[:, :], ones2_r[:], d2r[:, :], start=True, stop=True)
                bc_sb = p_bc.tile([P, 512], dt.float32, tag="bc", name="bc_sb")[:, :cw]
                nc.vector.tensor_copy(bc_sb[:, :], bp[:, :])
                bc_r = p_bc.tile([P, 512], dt.float32, tag="bcr", name="bc_r")[:, :cw]
                nc.vector.reciprocal(bc_r[:, :], bc_sb[:, :])
                ct = p_ctxT.tile([P, 512], dt.bfloat16, tag=f"ctxT{hp}", name="ct")[
                    :, :cw
                ]
                nc.vector.tensor_tensor(
                    ct[0:64, :], ctx_ps[0][0:DH, :cw], bc_r[0:64, :], ALU.mult
                )
                nc.vector.tensor_tensor(
                    ct[64:128, :], ctx_ps[1][0:DH, :cw], bc_r[64:128, :], ALU.mult
                )
                ctxT_sb[hp] = ct

            for qs in range(cw // P):
                o_sb = p_os.tile([P, D_MODEL], dt.float32, tag="osb", name="o_sb")
                for ft in range(2):
                    op = ps_mix.tile([P, 512], dt.float32, tag="psm", name="op")
                    for hp in range(2):
                        nc.tensor.matmul(
                            op[:, :],
                            ctxT_sb[hp][:, qs * P : (qs + 1) * P],
                            wo_sb[:, hp, ft * 512 : (ft + 1) * 512],
                            start=(hp == 0),
                            stop=(hp == 1),
                        )
                    nc.vector.tensor_copy(o_sb[:, ft * 512 : (ft + 1) * 512], op[:])
                nc.sync.dma_start(out[c0 + qs * P : c0 + (qs + 1) * P, :], o_sb[:])

    return nc


_cache = {}


def _get_program(LK):
    if LK not in _cache:
        _cache[LK] = _build(LK)
    return _cache[LK]


def kernel(**inputs) -> np.ndarray:
    import ml_dtypes

    x = np.asarray(inputs["x"], dtype=np.float32)
    rope_cos = np.asarray(inputs["rope_cos"], dtype=np.float32)
    rope_sin = np.asarray(inputs["rope_sin"], dtype=np.float32)
    L = np.asarray(inputs["input_lengths"]).astype(np.int64)
    Wq = np.asarray(inputs["Wq"], dtype=np.float32)
    Wk = np.asarray(inputs["Wk"], dtype=np.float32)
    Wv = np.asarray(inputs["Wv"], dtype=np.float32)
    Wo = np.asarray(inputs["Wo"], dtype=np.float32)
    bq = np.asarray(inputs["bq"], dtype=np.float32)
    bk = np.asarray(inputs["bk"], dtype=np.float32)
    bv = np.asarray(inputs["bv"], dtype=np.float32)
    bo = np.asarray(inputs["bo"], dtype=np.float32)

    Bn, Nn, Dn = x.shape
    assert (Bn, Nn, Dn) == (B, N, D_MODEL)
    assert not (np.any(bq) or np.any(bk) or np.any(bv)), (
        "nonzero qkv biases not supported by this kernel build"
    )

    Lmax = int(L.max())
    LK = max(((Lmax + P - 1) // P) * P, 256)
    KT = LK // P

    nc = _get_program(LK)

    sign = np.where(np.arange(CI) % 2 == 0, -1.0, 1.0).astype(np.float32)
    ones2 = np.zeros((2, P), np.float32)
    ones2[0, 0:64] = 1.0
    ones2[1, 64:128] = 1.0
    karr = np.arange(P)[:, None] + P * np.arange(KT)[None, :]

    in_maps = []
    for c in range(N_CORES):
        b, g = divmod(c, HPC)
        cols = slice(CI * g, CI * (g + 1))
        in_maps.append(
            {
                "xT": np.ascontiguousarray(x[b, :LK, :].T),
                "wq": np.ascontiguousarray(Wq[:, cols]),
                "wk": np.ascontiguousarray(Wk[:, cols]),
                "wv": np.ascontiguousarray(Wv[:, cols]),
                "wo": np.ascontiguousarray(Wo[cols, :]).astype(ml_dtypes.bfloat16),
                "cosT": np.ascontiguousarray(rope_cos[b, :LK, cols].T),
                "sinT": np.ascontiguousarray(rope_sin[b, :LK, cols].T * sign[:, None]),
                "vmask": (karr < L[b]).astype(np.float32),
                "ones2": ones2,
            }
        )

    res = run_bass_kernel_spmd(nc, in_maps, core_ids=list(range(N_CORES)))

    out = np.zeros((B, N, D_MODEL), np.float32)
    for c in range(N_CORES):
        b = c // HPC
        out[b, :LK, :] += res.results[c]["out"]
    for b in range(B):
        out[b, L[b] :, :] = 0.0
    out += bo[None, None, :]
    out *= (np.arange(N)[None, :] < L[:, None])[:, :, None].astype(np.float32)
    return out


# revision 22
# speedup vs baseline: 1.3305x; 1.0034x over previous
"""Multi-head attention (RoPE + length masking) on 8 Trainium2 NeuronCores.

Sharding: core c handles batch b = c // 4 and heads [4*(c%4), 4*(c%4)+4).
Each core computes q/k/v projections for its 256 inner dims, RoPE, per-head
attention with length masking, and a row-parallel slice of the output
projection; the host sums the 4 partial outputs per batch (all-reduce) and
applies the final key-mask zeroing.

Precision: float32r (reduced-mantissa fp32, full PE rate) for projections
and scores; bf16 for exp(scores), v, and the output projection. Measured
end-to-end relative error ~2e-3.
"""

import numpy as np
from contextlib import ExitStack

import bass_rust as _br
import concourse.bass as bass
import concourse.tile as tile
import concourse.mybir as mybir
from concourse.bass_utils import run_bass_kernel_spmd

P = 128
B, N, D_MODEL, H, DH = 2, 2048, 1024, 16, 64
N_CORES = 8
HPC = 4            # heads per core
CI = HPC * DH      # per-core inner dim (256)
KO = D_MODEL // P  # 8 contraction tiles

_counter = [0]


def _split_excess_waits(nc, max_normal=1, max_evsem=1):
    """Walrus in this toolchain rejects >1 semaphore wait per instruction.
    Hoist excess waits onto EventSemaphore carriers inserted right before
    the offending instruction in the same engine's program order."""
    for fn in nc.m.functions:
        for bb in fn.blocks:
            insts = bb.instructions
            out = []
            changed = False
            for inst in insts:
                si = inst.sync_info
                waits = list(si.on_wait or []) if si is not None else []
                cap = (
                    max_evsem
                    if isinstance(inst, mybir.InstEventSemaphore)
                    else max_normal
                )
                if len(waits) > cap:
                    extra, keep = waits[:-cap], waits[-cap:]
                    si.on_wait = keep
                    for i in range(0, len(extra), max_evsem):
                        _counter[0] += 1
                        ev = mybir.InstEventSemaphore(
                            name=f"I-waitsplit-{_counter[0]}",
                            engine=inst.engine,
                            sync_info=_br.SyncInfo(
                                on_wait=extra[i : i + max_evsem], on_update=[]
                            ),
                            ins=[],
                            outs=[],
                        )
                        out.append(ev)
                    changed = True
                out.append(inst)
            if changed:
                bb.instructions = out


class _TileContextFixed(tile.TileContext):
    def _drain_and_barrier(self, tick_clock, wait_clock):
        from concourse.tile import ScopedClock

        nc = self.nc
        drain_inst = nc.sync.drain()
        wait_clock.add_sem_waits(
            drain_inst.ins, ScopedClock({None: tick_clock.global_clock})
        )
        nc.all_engine_barrier()
        assert self.sems is not None
        popped = nc._tile_sem_poison_stack.pop()
        assert popped is self._sem_poison
        nc.clear_and_free_semaphores(list(self.sems.allocated().values()))
        nc.all_engine_barrier()
        _split_excess_waits(nc)


def _make_chunks(LK):
    """Split LK (multiple of 128) into free-dim chunks, preferring 512 and
    keeping every chunk >= 256 when possible (f32r full-rate needs >= 256)."""
    chunks = [512] * (LK // 512)
    rem = LK % 512
    if rem:
        if rem >= 256 or not chunks:
            chunks.append(rem)
        else:
            # e.g. rem=128 -> replace one 512 with 384 + 256
            chunks[-1] = 512 - (256 - rem)
            chunks.append(256)
    return chunks


def _build(LK):
    """Build the single-core Bass program (same program on all 8 cores)."""
    dt = mybir.dt
    KT = LK // P
    chunks = _make_chunks(LK)
    starts = np.cumsum([0] + chunks[:-1]).tolist()
    cidx = list(zip(chunks, starts))

    nc = bass.Bass(trn_type="TRN2")

    xT = nc.dram_tensor("xT", [D_MODEL, LK], dt.float32, kind="ExternalInput")
    wq = nc.dram_tensor("wq", [D_MODEL, CI], dt.float32, kind="ExternalInput")
    wk = nc.dram_tensor("wk", [D_MODEL, CI], dt.float32, kind="ExternalInput")
    wv = nc.dram_tensor("wv", [D_MODEL, CI], dt.float32, kind="ExternalInput")
    wo = nc.dram_tensor("wo", [CI, D_MODEL], dt.bfloat16, kind="ExternalInput")
    cosT = nc.dram_tensor("cosT", [CI, LK], dt.float16, kind="ExternalInput")
    sinT = nc.dram_tensor("sinT", [CI, LK], dt.float16, kind="ExternalInput")
    vmask = nc.dram_tensor("vmask", [P, KT], dt.float32, kind="ExternalInput")
    ones2 = nc.dram_tensor("ones2", [2, P], dt.float32, kind="ExternalInput")
    pswap = nc.dram_tensor("pswap", [P, P], dt.float32, kind="ExternalInput")
    out = nc.dram_tensor("out", [LK, D_MODEL], dt.float32, kind="ExternalOutput")

    F = mybir.ActivationFunctionType
    ALU = mybir.AluOpType

    with _TileContextFixed(nc) as tc, ExitStack() as ctx:
        # ---- long-lived pools ----
        p_qk = ctx.enter_context(tc.tile_pool(name="qk", bufs=1))
        p_v = ctx.enter_context(tc.tile_pool(name="v", bufs=1))
        p_const = ctx.enter_context(tc.tile_pool(name="const", bufs=1))

        qT = p_qk.tile([P, 2, LK], dt.float32r)
        kT = p_qk.tile([P, 2, LK], dt.float32r)
        v_sb = p_v.tile([P, KT, HPC, DH + 1], dt.bfloat16)
        wo_sb = p_const.tile([P, 2, D_MODEL], dt.bfloat16)
        vm_sb = p_const.tile([P, KT], dt.float32)
        ones2_r = p_const.tile([2, P], dt.float32r)

        nc.sync.dma_start(vm_sb[:], vmask[:])
        nc.sync.dma_start(wo_sb[:], wo.rearrange("(po pi) f -> pi po f", pi=P))
        ones2_f = p_const.tile([2, P], dt.float32)
        nc.sync.dma_start(ones2_f[:], ones2[:])
        nc.vector.tensor_copy(ones2_r[:], ones2_f[:])
        # ones column of v_aug carries the k-mask (0 rows beyond length)
        nc.vector.tensor_copy(
            v_sb[:, :, :, DH],
            vm_sb[:, :, None].to_broadcast((P, KT, HPC)),
        )

        # ---- phase A (scoped pools): load chunk-wise, project k, v, q ----
        with ExitStack() as actx:
            pa_w = actx.enter_context(tc.tile_pool(name="wstage", bufs=1))
            pa_x = actx.enter_context(tc.tile_pool(name="xtr", bufs=1))
            pa_stage = actx.enter_context(tc.tile_pool(name="stage", bufs=4))
            pa_tmp = actx.enter_context(tc.tile_pool(name="ropetmp", bufs=3))
            pa_cs = actx.enter_context(tc.tile_pool(name="cs", bufs=1))
            ps_proj = actx.enter_context(
                tc.tile_pool(name="psA", bufs=3, space="PSUM")
            )

            cos_sb = pa_cs.tile([P, 2, LK], dt.float16)
            sin_sb = pa_cs.tile([P, 2, LK], dt.float16)
            nc.sync.dma_start(cos_sb[:], cosT.rearrange("(po pi) s -> pi po s", pi=P))
            nc.sync.dma_start(sin_sb[:], sinT.rearrange("(po pi) s -> pi po s", pi=P))
            psw_f = pa_cs.tile([P, P], dt.float32)
            nc.sync.dma_start(psw_f[:], pswap[:])
            psw_r = pa_cs.tile([P, P], dt.float32r)
            nc.vector.tensor_copy(psw_r[:], psw_f[:])
            w_r = {}
            # k weights first: they gate the first projection matmuls
            wst_k = pa_stage.tile([P, KO, CI], dt.float32, tag="wst", name="wst_k")
            nc.sync.dma_start(wst_k[:], wk.rearrange("(ko ki) c -> ki ko c", ki=P))
            wr_k = pa_w.tile([P, KO, CI], dt.float32r, tag="wkr", name="wr_k")
            nc.vector.tensor_copy(wr_k[:], wst_k[:])
            w_r["k"] = wr_k

            xTr = pa_x.tile([P, KO, LK], dt.float32r)
            xT3 = xT.rearrange("(ko ki) s -> ki ko s", ki=P)
            for cw, c0 in cidx:
                for ko in range(KO):
                    xst = pa_stage.tile(
                        [P, 512], dt.float32, tag="xst", name="xst"
                    )[:, :cw]
                    nc.sync.dma_start(xst[:, :], xT3[:, ko, c0 : c0 + cw])
                    eng = nc.vector if ko % 2 == 0 else nc.gpsimd
                    eng.tensor_copy(xTr[:, ko, c0 : c0 + cw], xst[:, :])

            for name, wdram in (("v", wv), ("q", wq)):
                wst = pa_stage.tile([P, KO, CI], dt.float32, tag="wst", name="wst")
                nc.sync.dma_start(
                    wst[:], wdram.rearrange("(ko ki) c -> ki ko c", ki=P)
                )
                wr = pa_w.tile([P, KO, CI], dt.float32r, tag=f"w{name}", name="wr")
                nc.gpsimd.tensor_copy(wr[:], wst[:])
                w_r[name] = wr

            def project_rope(name, dst, p, cw, c0):
                pp = ps_proj.tile([P, 512], dt.float32, tag="psA", name="pp")[:, :cw]
                for ko in range(KO):
                    nc.tensor.matmul(
                        pp[:, :],
                        w_r[name][:, ko, p * P : (p + 1) * P],
                        xTr[:, ko, c0 : c0 + cw],
                        start=(ko == 0),
                        stop=(ko == KO - 1),
                    )
                t0 = pa_tmp.tile([P, 512], dt.float32r, tag="t0", name="t0")[:, :cw]
                nc.vector.tensor_copy(t0[:, :], pp[:, :])
                sw_ps = ps_proj.tile([P, 512], dt.float32, tag="psA", name="sw_ps")[
                    :, :cw
                ]
                nc.tensor.matmul(sw_ps[:, :], psw_r[:], t0[:, :], start=True, stop=True)
                t1 = pa_tmp.tile([P, 512], dt.float32, tag="t1", name="t1")[:, :cw]
                nc.vector.tensor_tensor(
                    t1[:, :], t0[:, :], cos_sb[:, p, c0 : c0 + cw], ALU.mult
                )
                t2 = pa_tmp.tile([P, 512], dt.float32, tag="t2", name="t2")[:, :cw]
                nc.vector.tensor_tensor(
                    t2[:, :], sw_ps[:, :], sin_sb[:, p, c0 : c0 + cw], ALU.mult
                )
                nc.vector.tensor_tensor(
                    dst[:, p, c0 : c0 + cw], t1[:, :], t2[:, :], ALU.add
                )

            for cw, c0 in cidx:
                for p in range(2):
                    project_rope("k", kT, p, cw, c0)
                for kt in range(c0 // P, (c0 + cw) // P):
                    vp = ps_proj.tile([P, 512], dt.float32, tag="psA", name="vp")[
                        :, :CI
                    ]
                    for ko in range(KO):
                        nc.tensor.matmul(
                            vp[:, :],
                            xTr[:, ko, kt * P : (kt + 1) * P],
                            w_r["v"][:, ko, :],
                            start=(ko == 0),
                            stop=(ko == KO - 1),
                        )
                    nc.vector.tensor_scalar_mul(
                        v_sb[:, kt, :, 0:DH],
                        vp[:, :].rearrange("p (h d) -> p h d", h=HPC),
                        vm_sb[:, kt : kt + 1],
                    )
            for cw, c0 in cidx:
                for p in range(2):
                    project_rope("q", qT, p, cw, c0)

        # ---- phases B+C per q-chunk ----
        p_e = ctx.enter_context(tc.tile_pool(name="e", bufs=4))
        p_ctxT = ctx.enter_context(tc.tile_pool(name="ctxT", bufs=2))
        p_rv = ctx.enter_context(tc.tile_pool(name="rv", bufs=2))
        p_bc = ctx.enter_context(tc.tile_pool(name="bc", bufs=2))
        p_os = ctx.enter_context(tc.tile_pool(name="os", bufs=3))
        ps_sc = ctx.enter_context(tc.tile_pool(name="psS", bufs=2, space="PSUM"))
        ps_ctx = ctx.enter_context(tc.tile_pool(name="psC", bufs=3, space="PSUM"))
        ps_mix = ctx.enter_context(tc.tile_pool(name="psM", bufs=1, space="PSUM"))

        kt_groups = [(k0, min(2, KT - k0)) for k0 in range(0, KT, 2)]

        for cw, c0 in cidx:
            ctxT_sb = {}
            for hp in range(2):
                ctx_ps = []
                for hi in range(2):
                    cp = ps_ctx.tile([P, 512], dt.float32, tag="ctxp", name="cp")
                    ctx_ps.append(cp)
                for k0, g in kt_groups:
                    sp2 = {}
                    for hi in range(2):
                        sp2[hi] = ps_sc.tile(
                            [P, 2, 512], dt.float32, tag="sc", name="sp2"
                        )
                    # scores: alternate heads so PE row-groups 0-63/64-127
                    # run concurrently
                    for j in range(g):
                        kt = k0 + j
                        for hi in range(2):
                            nc.tensor.matmul(
                                sp2[hi][:, j, :cw],
                                kT[64 * hi : 64 * hi + 64, hp, kt * P : (kt + 1) * P],
                                qT[64 * hi : 64 * hi + 64, hp, c0 : c0 + cw],
                                start=True,
                                stop=True,
                            )
                    for hi in range(2):
                        h = 2 * hp + hi
                        e_sb = p_e.tile([P, 2, 512], dt.bfloat16, tag="e", name="e_sb")
                        nc.scalar.activation(
                            e_sb[:, :g, :cw], sp2[hi][:, :g, :cw], F.Exp
                        )
                        for j in range(g):
                            kt = k0 + j
                            nc.tensor.matmul(
                                ctx_ps[hi][0 : DH + 1, :cw],
                                v_sb[:, kt, h, :],
                                e_sb[:, j, :cw],
                                start=(kt == 0),
                                stop=(kt == KT - 1),
                            )
                # denominator rows -> one [2,cw] tile -> block ones-matmul bcast
                d2f = p_rv.tile([2, 512], dt.float32, tag="d2f", name="d2f")[:, :cw]
                nc.vector.tensor_copy(d2f[0:1, :], ctx_ps[0][DH : DH + 1, :cw])
                d1 = p_rv.tile([1, 512], dt.float32, tag="d1", name="d1")[:, :cw]
                nc.vector.tensor_copy(d1[:, :], ctx_ps[1][DH : DH + 1, :cw])
                nc.sync.dma_start(d2f[1:2, :], d1[:, :])
                d2r = p_rv.tile([2, 512], dt.float32r, tag="d2r", name="d2r")[:, :cw]
                nc.vector.tensor_copy(d2r[:, :], d2f[:, :])
                bp = ps_mix.tile([P, 512], dt.float32, tag="psm", name="bp")[:, :cw]
                nc.tensor.matmul(bp[:, :], ones2_r[:], d2r[:, :], start=True, stop=True)
                bc_sb = p_bc.tile([P, 512], dt.float32, tag="bc", name="bc_sb")[:, :cw]
                nc.vector.tensor_copy(bc_sb[:, :], bp[:, :])
                bc_r = p_bc.tile([P, 512], dt.float32, tag="bcr", name="bc_r")[:, :cw]
                nc.vector.reciprocal(bc_r[:, :], bc_sb[:, :])
                ct = p_ctxT.tile([P, 512], dt.bfloat16, tag=f"ctxT{hp}", name="ct")[
                    :, :cw
                ]
                nc.vector.tensor_tensor(
                    ct[0:64, :], ctx_ps[0][0:DH, :cw], bc_r[0:64, :], ALU.mult
                )
                nc.vector.tensor_tensor(
                    ct[64:128, :], ctx_ps[1][0:DH, :cw], bc_r[64:128, :], ALU.mult
                )
                ctxT_sb[hp] = ct

            for qs in range(cw // P):
                o_sb = p_os.tile([P, D_MODEL], dt.float32, tag="osb", name="o_sb")
                for ft in range(2):
                    op = ps_mix.tile([P, 512], dt.float32, tag="psm", name="op")
                    for hp in range(2):
                        nc.tensor.matmul(
                            op[:, :],
                            ctxT_sb[hp][:, qs * P : (qs + 1) * P],
                            wo_sb[:, hp, ft * 512 : (ft + 1) * 512],
                            start=(hp == 0),
                            stop=(hp == 1),
                        )
                    nc.vector.tensor_copy(o_sb[:, ft * 512 : (ft + 1) * 512], op[:])
                nc.sync.dma_start(out[c0 + qs * P : c0 + (qs + 1) * P, :], o_sb[:])

    return nc


_cache = {}


def _get_program(LK):
    if LK not in _cache:
        _cache[LK] = _build(LK)
    return _cache[LK]


def kernel(**inputs) -> np.ndarray:
    import ml_dtypes

    x = np.asarray(inputs["x"], dtype=np.float32)
    rope_cos = np.asarray(inputs["rope_cos"], dtype=np.float32)
    rope_sin = np.asarray(inputs["rope_sin"], dtype=np.float32)
    L = np.asarray(inputs["input_lengths"]).astype(np.int64)
    Wq = np.asarray(inputs["Wq"], dtype=np.float32)
    Wk = np.asarray(inputs["Wk"], dtype=np.float32)
    Wv = np.asarray(inputs["Wv"], dtype=np.float32)
    Wo = np.asarray(inputs["Wo"], dtype=np.float32)
    bq = np.asarray(inputs["bq"], dtype=np.float32)
    bk = np.asarray(inputs["bk"], dtype=np.float32)
    bv = np.asarray(inputs["bv"], dtype=np.float32)
    bo = np.asarray(inputs["bo"], dtype=np.float32)

    Bn, Nn, Dn = x.shape
    assert (Bn, Nn, Dn) == (B, N, D_MODEL)
    assert not (np.any(bq) or np.any(bk) or np.any(bv)), (
        "nonzero qkv biases not supported by this kernel build"
    )

    Lmax = int(L.max())
    LK = max(((Lmax + P - 1) // P) * P, 256)
    KT = LK // P

    nc = _get_program(LK)

    sign = np.where(np.arange(CI) % 2 == 0, -1.0, 1.0).astype(np.float32)
    ones2 = np.zeros((2, P), np.float32)
    ones2[0, 0:64] = 1.0
    ones2[1, 64:128] = 1.0
    # pair-swap permutation: row d has a 1 in column (d^1); out = P_swap.T @ x
    pswap_m = np.zeros((P, P), np.float32)
    for d in range(P):
        pswap_m[d, d ^ 1] = 1.0
    karr = np.arange(P)[:, None] + P * np.arange(KT)[None, :]

    in_maps = []
    for c in range(N_CORES):
        b, g = divmod(c, HPC)
        cols = slice(CI * g, CI * (g + 1))
        in_maps.append(
            {
                "xT": np.ascontiguousarray(x[b, :LK, :].T),
                "wq": np.ascontiguousarray(Wq[:, cols]),
                "wk": np.ascontiguousarray(Wk[:, cols]),
                "wv": np.ascontiguousarray(Wv[:, cols]),
                "wo": np.ascontiguousarray(Wo[cols, :]).astype(ml_dtypes.bfloat16),
                "cosT": np.ascontiguousarray(rope_cos[b, :LK, cols].T).astype(np.float16),
                "sinT": np.ascontiguousarray(rope_sin[b, :LK, cols].T * sign[:, None]).astype(np.float16),
                "vmask": (karr < L[b]).astype(np.float32),
                "ones2": ones2,
                "pswap": pswap_m,
            }
        )

    res = run_bass_kernel_spmd(nc, in_maps, core_ids=list(range(N_CORES)))

    out = np.zeros((B, N, D_MODEL), np.float32)
    for c in range(N_CORES):
        b = c // HPC
        out[b, :LK, :] += res.results[c]["out"]
    for b in range(B):
        out[b, L[b] :, :] = 0.0
    out += bo[None, None, :]
    out *= (np.arange(N)[None, :] < L[:, None])[:, :, None].astype(np.float32)
    return out
